# revision 45
# baseline (speedup 1.0000x reference)
"""Trainium2 Bass kernel for nn_BGNLLLoss (bivariate-Gaussian NLL loss).

Math (per element t,p):
    mux,muy,lsx,lsy,pc = params[t,p,:];  x,y = targets[t,p,:]
    sx=e^lsx, sy=e^lsy, c=tanh(pc), nr=1-c^2
    a=(x-mux)/sx, b=(y-muy)/sy
    nll = min( (a^2+b^2-2abc)/(2nr) + lsx+lsy + 0.5 ln(nr) + ln(2pi),
               -ln(1e-20) )
    loss[p] = sum_t nll[t,p]

Reformulation used here (all engines stay in contiguous bf16):
  t4  = e^{-2 pc};  ah = (x-mux) e^{-lsx}/sqrt2;  bh = (y-muy) e^{-lsy}/sqrt2
  gs  = (ah+bh) t4 + (ah-bh);  gvs = gs e^{pc}/2      [= (a-cb)/sqrt(2nr)]
  W   = gvs^2 + bh^2                                  [= z/(2nr)]
  V   = W + (lsx+lsy-pc - ln(1+t4))                   [= nll - ln2 - ln2pi]
  nll = min(V + CADD, K) = K - relu((K-CADD) - V)
  loss[p] = T*K - sum_t relu((K-CADD) - V)            [relu'd row bounded ~50,
                                                       so bf16 sum is safe]

Input layout: host converts to bf16 planes and packs contiguous channel-group
row blocks (28 KiB/partition per block, 4 DMAs).  Engine split per 512-frame
block (8 blocks, software-pipelined):
  ScalarE: t4, isxh, isyh, sth (Exp), t4p1f (Identity), lvc (exponent-bits
           log), r2 = Relu((K-CADD) - V) read from PSUM (7 ACTIVATEs, one set)
  VectorE: 9 plain bf16 2x passes + 1 custom SQ2 (GpSimd is left idle on
           purpose: any GpSimd tensor op degrades VectorE via SBUF-port
           contention by more than the op is worth)
  TensorE: V = lsx+lsy-pc-lvc+W via accumulating identity matmuls ([I]/[-I]
           weights, partition=frame passthrough) into PSUM, plus the frame
           sum acc[1,512] += ones^T @ r2
Sharding: person dim split across 8 cores (512 each), no collectives.
Measured: 136.9us HW (baseline 324.6us); ScalarE 83% / VectorE 80% busy.
"""

import json
import math
import os
import shutil
import tempfile
from contextlib import ExitStack

import numpy as np
import ml_dtypes

import concourse.bass as bass
import concourse.bacc as bacc
import concourse.mybir as mybir
import concourse.tile as tile
from concourse import bass_utils
from concourse.dve_spec import Spec, Src0, Src1, sq, _has_src1
from concourse.dve_uop import DveOpSpec
import concourse.dve_ops as dve_ops

F32 = mybir.dt.float32
BF16 = mybir.dt.bfloat16
AF = mybir.ActivationFunctionType

T = 4096
P = 4096
N_CORES = 8
PC = P // N_CORES          # persons per core = 512
K = 4                      # 128-row subtiles per block
RB = 128 * K               # rows per block = 512
NB = T // RB               # 8 blocks
NCH = 7                    # x, y, mux, muy, lsx, lsy, pc
FD = K * PC                # free-dim elems per plane per block = 2048
ROW_W = NCH * FD           # bf16 elems per DMA row = 14336

LOG2PI = math.log(2.0 * math.pi)
LN2 = math.log(2.0)
CADD = LN2 + LOG2PI                    # nll = V + CADD before clamping
KCLAMP = -math.log(1e-20)              # 46.0517...
B_EXPH = -0.5 * LN2                    # exp bias: e^{-l}/sqrt2
B_STH = -LN2                           # exp bias: e^{pc}/2
B_LN1 = 1.0                            # identity bias: t4 + 1
B_RELU = KCLAMP - CADD                 # relu bias: relu(-V + (K-CADD))
TK_CONST = T * KCLAMP                  # loss = T*K - sum(r2)

# Fast-log constants: for x = 2^e (1+f), int_bits(x)/2^23 = e + 127 + f and
# ln(x) ~= (int_bits(x) - SIGMA) * ln2/2^23 with the mantissa correction
# c = 1.5 - 1/ln2 folded into SIGMA.  Keeps every ACTIVATE in the
# exp_and_others table set (no Ln set switch, ~2.6us/block saved).
LNK = math.log(2.0) / (1 << 23)
_C_MEAN = 1.5 - 1.0 / math.log(2.0)
SIGMA_F = (127.0 - _C_MEAN) * (1 << 23)
B_LVC = -SIGMA_F * LNK
LNK16 = math.log(2.0) / (1 << 7)       # bf16-bits variant (VectorE int16 TS)
B_LVC16 = -(127.0 - _C_MEAN) * math.log(2.0)

BF_NP = ml_dtypes.bfloat16


# --------------------------------------------------------------------------
# Custom DVE op: out = sq(in0) + sq(in1)  (one pass for gvs^2 + bh^2)
# --------------------------------------------------------------------------
def _register_dve_op(name: str, spec: Spec, subdim: bool = False):
    if name in dve_ops._SUB_OPCODE_FOR_NAME:
        return next(op for op in dve_ops.OPS if op.name == name)
    shas = {}
    for ver in ("v3", "v4"):
        uops = dve_spec_lower(spec, ver=ver)
        shas[ver] = DveOpSpec(
            name=name, opcode=0, uops=uops, rd1_en=_has_src1(spec)
        ).sha(ver)
    op = dve_ops.DveOp(name, spec, subdim=subdim, uops_sha=shas)
    dve_ops.OPS.append(op)
    dve_ops._SUB_OPCODE_FOR_NAME[name] = (
        dve_ops._CUSTOM_DVE_ROW_BASE + len(dve_ops.OPS) - 1
    )
    dve_ops.CUSTOM_DVE_SPECS[name] = spec
    return op


from concourse.dve_spec import lower as dve_spec_lower

SQ2 = _register_dve_op(
    "SQ2_BGNLL",
    Spec(
        body=sq(Src0) + sq(Src1),
        reference=lambda in0, in1, s0, s1, imm2: (
            np.square(in0.astype(np.float32)) + np.square(in1.astype(np.float32))
        ).astype(np.float32),
    ),
)


# --------------------------------------------------------------------------
# ACT table-set fix: walrus assigns Exp -> exp_and_others and Ln ->
# natural_log_exp_and_others, reloading tables every block (~2.6us/block).
# Reorder act_info.json so the combined exp+ln set is found first for both.
# --------------------------------------------------------------------------
def _install_act_json():
    if os.environ.get("BGNLL_NO_ACT_JSON"):
        return
    if os.environ.get("BASS_ACT_ROOT_JSON_PATH"):
        return
    try:
        from neuronxcc.driver.Job import Job
        from neuronxcc.driver.jobs.support.FindActInfo import findActInfoFile
        src = findActInfoFile(Job.getPackageDir(), "gen3")
    except Exception:
        return
    if not src:
        return
    src_dir = os.path.dirname(src)
    dst_dir = os.path.join(tempfile.gettempdir(), "bgnll_act_root")
    os.makedirs(dst_dir, exist_ok=True)
    with open(src) as f:
        info = json.load(f)
    sets = info.get("act_func_sets", [])
    pref = [s for s in sets if s.get("name") == "natural_log_exp_and_others"]
    rest = [s for s in sets if s.get("name") != "natural_log_exp_and_others"]
    if not pref:
        return
    info["act_func_sets"] = pref + rest
    for name in os.listdir(src_dir):
        s = os.path.join(src_dir, name)
        d = os.path.join(dst_dir, name)
        if os.path.isfile(s) and not os.path.exists(d) and name != "act_info.json":
            try:
                os.symlink(s, d)
            except OSError:
                shutil.copy(s, d)
    with open(os.path.join(dst_dir, "act_info.json"), "w") as f:
        json.dump(info, f)
    os.environ["BASS_ACT_ROOT_JSON_PATH"] = os.path.join(dst_dir, "act_info.json")


# --------------------------------------------------------------------------
# Kernel body (per core; SPMD -- same program on all 8 cores)
# --------------------------------------------------------------------------
def _emit(ctx: ExitStack, tc: tile.TileContext, inp: bass.AP, eye_in: bass.AP,
          loss: bass.AP):
    nc = tc.nc

    iot = ctx.enter_context(tc.tile_pool(name="iot", bufs=3))
    tp = ctx.enter_context(tc.tile_pool(name="tp", bufs=2))
    single = ctx.enter_context(tc.tile_pool(name="single", bufs=1))
    psum_pool = ctx.enter_context(
        tc.tile_pool(name="psum", bufs=1, space="PSUM")
    )

    ones = single.tile([128, 1], BF16)
    nc.vector.memset(ones[:], 1.0)
    acc = psum_pool.tile([1, PC], F32)
    # [I | -I] identity weights: TensorE computes the log-det side chain
    # s1bl = lsx + lsy - pc - lvc as 4 accumulating identity matmuls per
    # k-subtile into PSUM (partition=frame layout makes w=I a passthrough),
    # freeing 3 VectorE passes per block.
    eyeP = single.tile([128, 128], BF16)
    eyeM = single.tile([128, 128], BF16)
    nc.sync.dma_start(eyeP[:], eye_in[0])
    nc.sync.dma_start(eyeM[:], eye_in[1])
    sb1 = psum_pool.tile([128, K, PC], F32)

    shf = [128, FD]
    ctxs: dict[int, dict] = {}

    # Channel-group split DMA: 4 transfers per block ({x,y}, {mux,muy},
    # {lsx,lsy}, {pc}) so each consumer stage starts as soon as its channels
    # land, shrinking the pipeline fill and smoothing DMA bursts.
    GRP = [(0, 2), (2, 2), (4, 2), (6, 1)]

    def stage_load(blk):
        rows = inp[blk * 128:(blk + 1) * 128, :]
        tiles = []
        for gi, (c0, ncg) in enumerate(GRP):
            tg = iot.tile([128, ncg, K, PC], BF16, tag=f"in{gi}")
            nc.sync.dma_start(
                tg[:].rearrange("p c k n -> p (c k n)"),
                rows[:, c0 * FD:(c0 + ncg) * FD])
            tiles.append(tg)
        ctxs[blk] = {"in": tiles}

    def chv(c, i):
        for gi, (c0, ncg) in enumerate(GRP):
            if c0 <= i < c0 + ncg:
                return c["in"][gi][:, i - c0].rearrange("p k n -> p (k n)")
        raise AssertionError(i)

    def stage_front(blk):
        c = ctxs[blk]
        xv, yv = chv(c, 0), chv(c, 1)
        mxv, myv = chv(c, 2), chv(c, 3)
        lxv, lyv, pcv = chv(c, 4), chv(c, 5), chv(c, 6)

        t4 = tp.tile(shf, BF16, tag="t4")
        t4p1 = tp.tile([128, K, PC], BF16, tag="t4p1")
        isxh = tp.tile(shf, BF16, tag="isxh")
        isyh = tp.tile(shf, BF16, tag="isyh")
        sth = tp.tile(shf, BF16, tag="sth")
        lvc = tp.tile([128, K, PC], BF16, tag="lvc")
        c.update(t4=t4, isxh=isxh, isyh=isyh, sth=sth, lvc=lvc,
                 xv=xv, yv=yv, mxv=mxv, myv=myv)

        # --- ScalarE: single table set (exp_and_others only) ---
        nc.scalar.activation(t4[:], pcv, AF.Exp, scale=-2.0)
        nc.scalar.activation(isxh[:], lxv, AF.Exp, scale=-1.0, bias=B_EXPH)
        nc.scalar.activation(isyh[:], lyv, AF.Exp, scale=-1.0, bias=B_EXPH)
        nc.scalar.activation(sth[:], pcv, AF.Exp, scale=1.0, bias=B_STH)
        nc.scalar.activation(t4p1[:].rearrange("p k n -> p (k n)"), t4[:],
                             AF.Identity, scale=1.0, bias=B_LN1)
        c["t4p1"] = t4p1

        c["lx3"] = c["in"][2][:, 0]
        c["ly3"] = c["in"][2][:, 1]
        c["pc3"] = c["in"][3][:, 0]

    def stage_dve(blk):
        c = ctxs[blk]
        A = tp.tile(shf, BF16, tag="A")      # nxt -> ah -> dab -> ...
        B = tp.tile(shf, BF16, tag="B")      # nyt -> bh
        C = tp.tile([128, K, PC], BF16, tag="C")  # sab->m1->gs->gvs->W
        Cf = C[:].rearrange("p k n -> p (k n)")
        r2 = tp.tile([128, K, PC], BF16, tag="r2")

        # --- TensorE: s1bl = lsx + lsy - pc - lvc via identity matmuls.
        # partition=frame, so w=I passes each [128,512] k-subtile through;
        # PSUM accumulates the four terms in fp32.  Emitted in the same wave
        # as the consuming V pass so the single-buffered PSUM tile has no
        # cross-wave WAR hazard.
        lvc = c["lvc"]
        # lvc = ln(1+t4) via the bf16 exponent-bits log on VectorE (int16
        # tensor_scalar) — ScalarE is the wall.  Its matmul contribution is
        # deferred to the END of each bank's accumulation group, so the
        # s1bl matmuls below never wait on it (this ordering is what round-3
        # lacked: lvc must not gate the start of the PSUM group).
        nc.vector.tensor_scalar(
            lvc[:].rearrange("p k n -> p (k n)"),
            c["t4p1"][:].rearrange("p k n -> p (k n)").bitcast(mybir.dt.int16),
            LNK16, B_LVC16, mybir.AluOpType.mult, mybir.AluOpType.add)
        for k in range(K):
            nc.tensor.matmul(sb1[:, k, :], eyeP[:], c["lx3"][:, k, :],
                             start=True, stop=False)
            nc.tensor.matmul(sb1[:, k, :], eyeP[:], c["ly3"][:, k, :],
                             start=False, stop=False)
            nc.tensor.matmul(sb1[:, k, :], eyeM[:], c["pc3"][:, k, :],
                             start=False, stop=False)

        nc.vector.tensor_sub(A[:], c["xv"], c["mxv"])         # nxt
        nc.vector.tensor_sub(B[:], c["yv"], c["myv"])         # nyt
        nc.vector.tensor_mul(A[:], A[:], c["isxh"][:])        # ah
        nc.vector.tensor_mul(B[:], B[:], c["isyh"][:])        # bh
        nc.vector.tensor_add(Cf, A[:], B[:])                  # sab
        nc.vector.tensor_sub(A[:], A[:], B[:])                # dab
        nc.vector.tensor_mul(Cf, Cf, c["t4"][:])              # m1
        nc.vector.tensor_add(Cf, Cf, A[:])                    # gs
        nc.vector.tensor_mul(Cf, Cf, c["sth"][:])             # gvs
        nc.vector._custom_dve(SQ2, out=Cf, in0=Cf, in1=B[:])  # W
        # V finishes inside PSUM: W then lvc per bank; lvc carries stop=True
        for k in range(K):
            nc.tensor.matmul(sb1[:, k, :], eyeP[:], C[:, k, :],
                             start=False, stop=False)
            nc.tensor.matmul(sb1[:, k, :], eyeM[:], lvc[:, k, :],
                             start=False, stop=True)

        # r2 = relu((K-CADD) - V); ScalarE reads V straight from PSUM
        nc.scalar.activation(r2[:].rearrange("p k n -> p (k n)"),
                             sb1[:].rearrange("p k n -> p (k n)"),
                             AF.Relu, scale=-1.0, bias=B_RELU)

        # --- TensorE: frame sum of r2 ---
        for k in range(K):
            nc.tensor.matmul(
                acc[:, :], ones[:, :], r2[:, k, :],
                start=(blk == 0 and k == 0),
                stop=(blk == NB - 1 and k == K - 1),
            )
        del ctxs[blk]

    # Skewed emission (software pipelining): DMA for blk+2, producers for
    # blk+1, consumers for blk.
    for i in range(NB + 2):
        if i < NB:
            stage_load(i)
        if 1 <= i and i - 1 < NB:
            stage_front(i - 1)
        if 2 <= i and i - 2 < NB:
            stage_dve(i - 2)

    # loss = T*K - acc
    tk = single.tile([1, PC], F32)
    nc.vector.memset(tk[:], TK_CONST)
    out_sb = single.tile([1, PC], F32)
    nc.vector.tensor_sub(out_sb[:], tk[:], acc[:, :])
    nc.sync.dma_start(loss, out_sb[:])


_CACHED_NC = None


def _build_program() -> bass.Bass:
    global _CACHED_NC
    if _CACHED_NC is not None:
        return _CACHED_NC
    # NOTE: _install_act_json (baseline's exp+ln table-set reorder) crashed
    # at runtime on HW; the kernel now keeps every ACTIVATE in the default
    # exp_and_others set instead, so it is intentionally NOT called.
    nc = bacc.Bacc("TRN2", target_bir_lowering=False, debug=False,
                   enable_asserts=False)
    for v in (B_EXPH, B_STH, B_LN1, B_RELU, B_LVC):
        t = nc.alloc_sbuf_tensor(f"const-f32-{v}", [128, 1], F32)
        nc.gpsimd.memset(t.ap(), v)
        nc.const_aps.aps[(F32, v)] = t.ap()
    nc.all_engine_barrier()
    inp = nc.dram_tensor("inp", [NB * 128, ROW_W], BF16,
                         kind="ExternalInput").ap()
    eye_p = nc.dram_tensor("eyep", [128, 128], BF16,
                           kind="ExternalInput").ap()
    eye_m = nc.dram_tensor("eyem", [128, 128], BF16,
                           kind="ExternalInput").ap()
    eye_in = (eye_p, eye_m)
    loss = nc.dram_tensor("loss", [1, PC], F32, kind="ExternalOutput").ap()
    with tile.TileContext(nc) as tc:
        with ExitStack() as ctx:
            _emit(ctx, tc, inp, eye_in, loss)
    nc.compile()
    _CACHED_NC = nc
    return nc


def make_in_maps(targets: np.ndarray, params: np.ndarray):
    targets = np.asarray(targets, dtype=np.float32)
    params = np.asarray(params, dtype=np.float32)
    in_maps = []
    for i in range(N_CORES):
        sl = slice(i * PC, (i + 1) * PC)
        # planes: x, y, mux, muy, lsx, lsy, pc  -> [T, NCH, PC] bf16
        pl = np.concatenate(
            [targets[:, sl, :].transpose(0, 2, 1),
             params[:, sl, :].transpose(0, 2, 1)], axis=1
        ).astype(BF_NP)                          # [T, 7, 512]
        # row t = blk*512 + k*128 + p  ->  [NB, 128, 7, K, 512]
        pl = pl.reshape(NB, K, 128, NCH, PC).transpose(0, 2, 3, 1, 4)
        in_maps.append({
            "inp": np.ascontiguousarray(pl).reshape(NB * 128, ROW_W),
            "eyep": np.eye(128, dtype=np.float32).astype(BF_NP),
            "eyem": (-np.eye(128, dtype=np.float32)).astype(BF_NP),
        })
    return in_maps


def run_spmd(targets: np.ndarray, params: np.ndarray, trace: bool = False):
    nc = _build_program()
    in_maps = make_in_maps(targets, params)
    res = bass_utils.run_bass_kernel_spmd(
        nc, in_maps, core_ids=list(range(N_CORES)), trace=trace,
    )
    loss = np.concatenate(
        [res.results[i]["loss"].reshape(PC) for i in range(N_CORES)]
    ).astype(np.float32)
    return loss, res


def kernel(targets: np.ndarray, params: np.ndarray,
           peopleIDs: np.ndarray | None = None) -> np.ndarray:
    loss, _ = run_spmd(targets, params, trace=False)
    return loss


# revision 48
# speedup vs baseline: 1.0267x; 1.0267x over previous
"""Trainium2 Bass kernel for nn_BGNLLLoss (bivariate-Gaussian NLL loss).

Math (per element t,p):
    mux,muy,lsx,lsy,pc = params[t,p,:];  x,y = targets[t,p,:]
    sx=e^lsx, sy=e^lsy, c=tanh(pc), nr=1-c^2
    a=(x-mux)/sx, b=(y-muy)/sy
    nll = min( (a^2+b^2-2abc)/(2nr) + lsx+lsy + 0.5 ln(nr) + ln(2pi),
               -ln(1e-20) )
    loss[p] = sum_t nll[t,p]

Reformulation used here (all engines stay in contiguous bf16):
  t4  = e^{-2 pc};  ah = (x-mux) e^{-lsx}/sqrt2;  bh = (y-muy) e^{-lsy}/sqrt2
  gs  = (ah+bh) t4 + (ah-bh);  gvs = gs e^{pc}/2      [= (a-cb)/sqrt(2nr)]
  W   = gvs^2 + bh^2                                  [= z/(2nr)]
  V   = W + (lsx+lsy-pc - ln(1+t4))                   [= nll - ln2 - ln2pi]
  nll = min(V + CADD, K) = K - relu((K-CADD) - V)
  loss[p] = T*K - sum_t relu((K-CADD) - V)            [relu'd row bounded ~50,
                                                       so bf16 sum is safe]

Input layout: host converts to bf16 planes and packs contiguous channel-group
row blocks (28 KiB/partition per block, 4 DMAs).  Engine split per 512-frame
block (8 blocks, software-pipelined):
  ScalarE: t4, isxh, isyh, sth (Exp), t4p1f (Identity), lvc (exponent-bits
           log), r2 = Relu((K-CADD) - V) read from PSUM (7 ACTIVATEs, one set)
  VectorE: 9 plain bf16 2x passes + 1 custom SQ2 (GpSimd is left idle on
           purpose: any GpSimd tensor op degrades VectorE via SBUF-port
           contention by more than the op is worth)
  TensorE: V = lsx+lsy-pc-lvc+W via accumulating identity matmuls ([I]/[-I]
           weights, partition=frame passthrough) into PSUM, plus the frame
           sum acc[1,512] += ones^T @ r2
Sharding: person dim split across 8 cores (512 each), no collectives.
Measured: 136.9us HW (baseline 324.6us); ScalarE 83% / VectorE 80% busy.
"""

import json
import math
import os
import shutil
import tempfile
from contextlib import ExitStack

import numpy as np
import ml_dtypes

import concourse.bass as bass
import concourse.bacc as bacc
import concourse.mybir as mybir
import concourse.tile as tile
from concourse import bass_utils
from concourse.dve_spec import Spec, Src0, Src1, sq, _has_src1
from concourse.dve_uop import DveOpSpec
import concourse.dve_ops as dve_ops

F32 = mybir.dt.float32
BF16 = mybir.dt.bfloat16
AF = mybir.ActivationFunctionType

T = 4096
P = 4096
N_CORES = 8
PC = P // N_CORES          # persons per core = 512
K = 4                      # 128-row subtiles per block
RB = 128 * K               # rows per block = 512
NB = T // RB               # 8 blocks
NCH = 7                    # x, y, mux, muy, lsx, lsy, pc
FD = K * PC                # free-dim elems per plane per block = 2048
ROW_W = NCH * FD           # bf16 elems per DMA row = 14336

LOG2PI = math.log(2.0 * math.pi)
LN2 = math.log(2.0)
CADD = LN2 + LOG2PI                    # nll = V + CADD before clamping
KCLAMP = -math.log(1e-20)              # 46.0517...
B_EXPH = -0.5 * LN2                    # exp bias: e^{-l}/sqrt2
B_STH = -LN2                           # exp bias: e^{pc}/2
B_LN1 = 1.0                            # identity bias: t4 + 1
B_RELU = KCLAMP - CADD                 # relu bias: relu(-V + (K-CADD))
TK_CONST = T * KCLAMP                  # loss = T*K - sum(r2)

# Fast-log constants: for x = 2^e (1+f), int_bits(x)/2^23 = e + 127 + f and
# ln(x) ~= (int_bits(x) - SIGMA) * ln2/2^23 with the mantissa correction
# c = 1.5 - 1/ln2 folded into SIGMA.  Keeps every ACTIVATE in the
# exp_and_others table set (no Ln set switch, ~2.6us/block saved).
LNK = math.log(2.0) / (1 << 23)
_C_MEAN = 1.5 - 1.0 / math.log(2.0)
SIGMA_F = (127.0 - _C_MEAN) * (1 << 23)
B_LVC = -SIGMA_F * LNK

BF_NP = ml_dtypes.bfloat16


# --------------------------------------------------------------------------
# Custom DVE op: out = sq(in0) + sq(in1)  (one pass for gvs^2 + bh^2)
# --------------------------------------------------------------------------
def _register_dve_op(name: str, spec: Spec, subdim: bool = False):
    if name in dve_ops._SUB_OPCODE_FOR_NAME:
        return next(op for op in dve_ops.OPS if op.name == name)
    shas = {}
    for ver in ("v3", "v4"):
        uops = dve_spec_lower(spec, ver=ver)
        shas[ver] = DveOpSpec(
            name=name, opcode=0, uops=uops, rd1_en=_has_src1(spec)
        ).sha(ver)
    op = dve_ops.DveOp(name, spec, subdim=subdim, uops_sha=shas)
    dve_ops.OPS.append(op)
    dve_ops._SUB_OPCODE_FOR_NAME[name] = (
        dve_ops._CUSTOM_DVE_ROW_BASE + len(dve_ops.OPS) - 1
    )
    dve_ops.CUSTOM_DVE_SPECS[name] = spec
    return op


from concourse.dve_spec import lower as dve_spec_lower

SQ2 = _register_dve_op(
    "SQ2_BGNLL",
    Spec(
        body=sq(Src0) + sq(Src1),
        reference=lambda in0, in1, s0, s1, imm2: (
            np.square(in0.astype(np.float32)) + np.square(in1.astype(np.float32))
        ).astype(np.float32),
    ),
)


# --------------------------------------------------------------------------
# ACT table-set fix: walrus assigns Exp -> exp_and_others and Ln ->
# natural_log_exp_and_others, reloading tables every block (~2.6us/block).
# Reorder act_info.json so the combined exp+ln set is found first for both.
# --------------------------------------------------------------------------
def _install_act_json():
    if os.environ.get("BGNLL_NO_ACT_JSON"):
        return
    if os.environ.get("BASS_ACT_ROOT_JSON_PATH"):
        return
    try:
        from neuronxcc.driver.Job import Job
        from neuronxcc.driver.jobs.support.FindActInfo import findActInfoFile
        src = findActInfoFile(Job.getPackageDir(), "gen3")
    except Exception:
        return
    if not src:
        return
    src_dir = os.path.dirname(src)
    dst_dir = os.path.join(tempfile.gettempdir(), "bgnll_act_root")
    os.makedirs(dst_dir, exist_ok=True)
    with open(src) as f:
        info = json.load(f)
    sets = info.get("act_func_sets", [])
    pref = [s for s in sets if s.get("name") == "natural_log_exp_and_others"]
    rest = [s for s in sets if s.get("name") != "natural_log_exp_and_others"]
    if not pref:
        return
    info["act_func_sets"] = pref + rest
    for name in os.listdir(src_dir):
        s = os.path.join(src_dir, name)
        d = os.path.join(dst_dir, name)
        if os.path.isfile(s) and not os.path.exists(d) and name != "act_info.json":
            try:
                os.symlink(s, d)
            except OSError:
                shutil.copy(s, d)
    with open(os.path.join(dst_dir, "act_info.json"), "w") as f:
        json.dump(info, f)
    os.environ["BASS_ACT_ROOT_JSON_PATH"] = os.path.join(dst_dir, "act_info.json")


# --------------------------------------------------------------------------
# Kernel body (per core; SPMD -- same program on all 8 cores)
# --------------------------------------------------------------------------
def _emit(ctx: ExitStack, tc: tile.TileContext, inp: bass.AP, eye_in: bass.AP,
          loss: bass.AP):
    nc = tc.nc

    iot = ctx.enter_context(tc.tile_pool(name="iot", bufs=3))
    tp = ctx.enter_context(tc.tile_pool(name="tp", bufs=2))
    single = ctx.enter_context(tc.tile_pool(name="single", bufs=1))
    psum_pool = ctx.enter_context(
        tc.tile_pool(name="psum", bufs=1, space="PSUM")
    )

    ones = single.tile([128, 1], BF16)
    nc.vector.memset(ones[:], 1.0)
    acc = psum_pool.tile([1, PC], F32)
    # [I | -I] identity weights: TensorE computes the log-det side chain
    # s1bl = lsx + lsy - pc - lvc as 4 accumulating identity matmuls per
    # k-subtile into PSUM (partition=frame layout makes w=I a passthrough),
    # freeing 3 VectorE passes per block.
    eyeP = single.tile([128, 128], BF16)
    eyeM = single.tile([128, 128], BF16)
    nc.sync.dma_start(eyeP[:], eye_in[0])
    nc.sync.dma_start(eyeM[:], eye_in[1])
    sb1 = psum_pool.tile([128, K, PC], F32)

    shf = [128, FD]
    ctxs: dict[int, dict] = {}

    # Channel-group split DMA: 4 transfers per block ({x,y}, {mux,muy},
    # {lsx,lsy}, {pc}) so each consumer stage starts as soon as its channels
    # land, shrinking the pipeline fill and smoothing DMA bursts.
    GRP = [(0, 2), (2, 2), (4, 2), (6, 1)]

    def stage_load(blk):
        rows = inp[blk * 128:(blk + 1) * 128, :]
        tiles = []
        for gi, (c0, ncg) in enumerate(GRP):
            tg = iot.tile([128, ncg, K, PC], BF16, tag=f"in{gi}")
            nc.sync.dma_start(
                tg[:].rearrange("p c k n -> p (c k n)"),
                rows[:, c0 * FD:(c0 + ncg) * FD])
            tiles.append(tg)
        ctxs[blk] = {"in": tiles}

    def chv(c, i):
        for gi, (c0, ncg) in enumerate(GRP):
            if c0 <= i < c0 + ncg:
                return c["in"][gi][:, i - c0].rearrange("p k n -> p (k n)")
        raise AssertionError(i)

    def stage_front(blk):
        c = ctxs[blk]
        xv, yv = chv(c, 0), chv(c, 1)
        mxv, myv = chv(c, 2), chv(c, 3)
        lxv, lyv, pcv = chv(c, 4), chv(c, 5), chv(c, 6)

        t4 = tp.tile(shf, BF16, tag="t4")
        t4p1f = tp.tile(shf, F32, tag="t4p1f")
        isxh = tp.tile(shf, BF16, tag="isxh")
        isyh = tp.tile(shf, BF16, tag="isyh")
        sth = tp.tile(shf, BF16, tag="sth")
        lvc = tp.tile([128, K, PC], BF16, tag="lvc")
        c.update(t4=t4, isxh=isxh, isyh=isyh, sth=sth, lvc=lvc,
                 xv=xv, yv=yv, mxv=mxv, myv=myv)

        # --- ScalarE: single table set (exp_and_others only) ---
        nc.scalar.activation(t4[:], pcv, AF.Exp, scale=-2.0)
        nc.scalar.activation(isxh[:], lxv, AF.Exp, scale=-1.0, bias=B_EXPH)
        nc.scalar.activation(isyh[:], lyv, AF.Exp, scale=-1.0, bias=B_EXPH)
        nc.scalar.activation(sth[:], pcv, AF.Exp, scale=1.0, bias=B_STH)
        nc.scalar.activation(t4p1f[:], t4[:], AF.Identity, scale=1.0,
                             bias=B_LN1)
        # lvc = ln(1+t4) via the exponent-bits log approximation
        nc.scalar.activation(lvc[:].rearrange("p k n -> p (k n)"),
                             t4p1f[:].bitcast(mybir.dt.int32),
                             AF.Identity, scale=LNK, bias=B_LVC)

        c["lx3"] = c["in"][2][:, 0]
        c["ly3"] = c["in"][2][:, 1]
        c["pc3"] = c["in"][3][:, 0]

    def stage_dve(blk):
        c = ctxs[blk]
        A = tp.tile(shf, BF16, tag="A")      # nxt -> ah -> dab -> ...
        B = tp.tile(shf, BF16, tag="B")      # nyt -> bh
        C = tp.tile([128, K, PC], BF16, tag="C")  # sab->m1->gs->gvs->W
        Cf = C[:].rearrange("p k n -> p (k n)")
        r2 = tp.tile([128, K, PC], BF16, tag="r2")

        # --- TensorE: s1bl = lsx + lsy - pc - lvc via identity matmuls.
        # partition=frame, so w=I passes each [128,512] k-subtile through;
        # PSUM accumulates the four terms in fp32.  Emitted in the same wave
        # as the consuming V pass so the single-buffered PSUM tile has no
        # cross-wave WAR hazard.
        lvc = c["lvc"]
        for k in range(K):
            nc.tensor.matmul(sb1[:, k, :], eyeP[:], c["lx3"][:, k, :],
                             start=True, stop=False)
            nc.tensor.matmul(sb1[:, k, :], eyeP[:], c["ly3"][:, k, :],
                             start=False, stop=False)
            nc.tensor.matmul(sb1[:, k, :], eyeM[:], c["pc3"][:, k, :],
                             start=False, stop=False)
            nc.tensor.matmul(sb1[:, k, :], eyeM[:], lvc[:, k, :],
                             start=False, stop=False)

        nc.vector.tensor_sub(A[:], c["xv"], c["mxv"])         # nxt
        nc.vector.tensor_sub(B[:], c["yv"], c["myv"])         # nyt
        nc.vector.tensor_mul(A[:], A[:], c["isxh"][:])        # ah
        nc.vector.tensor_mul(B[:], B[:], c["isyh"][:])        # bh
        nc.vector.tensor_add(Cf, A[:], B[:])                  # sab
        nc.vector.tensor_sub(A[:], A[:], B[:])                # dab
        nc.vector.tensor_mul(Cf, Cf, c["t4"][:])              # m1
        nc.vector.tensor_add(Cf, Cf, A[:])                    # gs
        nc.vector.tensor_mul(Cf, Cf, c["sth"][:])             # gvs
        nc.vector._custom_dve(SQ2, out=Cf, in0=Cf, in1=B[:])  # W
        # V = W + s1bl finishes inside PSUM: 4 more identity matmuls add W
        for k in range(K):
            nc.tensor.matmul(sb1[:, k, :], eyeP[:], C[:, k, :],
                             start=False, stop=True)

        # r2 = relu((K-CADD) - V); ScalarE reads V straight from PSUM
        nc.scalar.activation(r2[:].rearrange("p k n -> p (k n)"),
                             sb1[:].rearrange("p k n -> p (k n)"),
                             AF.Relu, scale=-1.0, bias=B_RELU)

        # --- TensorE: frame sum of r2 ---
        for k in range(K):
            nc.tensor.matmul(
                acc[:, :], ones[:, :], r2[:, k, :],
                start=(blk == 0 and k == 0),
                stop=(blk == NB - 1 and k == K - 1),
            )
        del ctxs[blk]

    # Skewed emission (software pipelining): DMA for blk+2, producers for
    # blk+1, consumers for blk.
    for i in range(NB + 2):
        if i < NB:
            stage_load(i)
        if 1 <= i and i - 1 < NB:
            stage_front(i - 1)
        if 2 <= i and i - 2 < NB:
            stage_dve(i - 2)

    # loss = T*K - acc
    tk = single.tile([1, PC], F32)
    nc.vector.memset(tk[:], TK_CONST)
    out_sb = single.tile([1, PC], F32)
    nc.vector.tensor_sub(out_sb[:], tk[:], acc[:, :])
    nc.sync.dma_start(loss, out_sb[:])


_CACHED_NC = None


def _build_program() -> bass.Bass:
    global _CACHED_NC
    if _CACHED_NC is not None:
        return _CACHED_NC
    # NOTE: _install_act_json (baseline's exp+ln table-set reorder) crashed
    # at runtime on HW; the kernel now keeps every ACTIVATE in the default
    # exp_and_others set instead, so it is intentionally NOT called.
    nc = bacc.Bacc("TRN2", target_bir_lowering=False, debug=False,
                   enable_asserts=False)
    for v in (B_EXPH, B_STH, B_LN1, B_RELU, B_LVC):
        t = nc.alloc_sbuf_tensor(f"const-f32-{v}", [128, 1], F32)
        nc.gpsimd.memset(t.ap(), v)
        nc.const_aps.aps[(F32, v)] = t.ap()
    nc.all_engine_barrier()
    inp = nc.dram_tensor("inp", [NB * 128, ROW_W], BF16,
                         kind="ExternalInput").ap()
    eye_p = nc.dram_tensor("eyep", [128, 128], BF16,
                           kind="ExternalInput").ap()
    eye_m = nc.dram_tensor("eyem", [128, 128], BF16,
                           kind="ExternalInput").ap()
    eye_in = (eye_p, eye_m)
    loss = nc.dram_tensor("loss", [1, PC], F32, kind="ExternalOutput").ap()
    with tile.TileContext(nc) as tc:
        with ExitStack() as ctx:
            _emit(ctx, tc, inp, eye_in, loss)
    nc.compile()
    _CACHED_NC = nc
    return nc


def make_in_maps(targets: np.ndarray, params: np.ndarray):
    targets = np.asarray(targets, dtype=np.float32)
    params = np.asarray(params, dtype=np.float32)
    in_maps = []
    for i in range(N_CORES):
        sl = slice(i * PC, (i + 1) * PC)
        # planes: x, y, mux, muy, lsx, lsy, pc  -> [T, NCH, PC] bf16
        pl = np.concatenate(
            [targets[:, sl, :].transpose(0, 2, 1),
             params[:, sl, :].transpose(0, 2, 1)], axis=1
        ).astype(BF_NP)                          # [T, 7, 512]
        # row t = blk*512 + k*128 + p  ->  [NB, 128, 7, K, 512]
        pl = pl.reshape(NB, K, 128, NCH, PC).transpose(0, 2, 3, 1, 4)
        in_maps.append({
            "inp": np.ascontiguousarray(pl).reshape(NB * 128, ROW_W),
            "eyep": np.eye(128, dtype=np.float32).astype(BF_NP),
            "eyem": (-np.eye(128, dtype=np.float32)).astype(BF_NP),
        })
    return in_maps


def run_spmd(targets: np.ndarray, params: np.ndarray, trace: bool = False):
    nc = _build_program()
    in_maps = make_in_maps(targets, params)
    res = bass_utils.run_bass_kernel_spmd(
        nc, in_maps, core_ids=list(range(N_CORES)), trace=trace,
    )
    loss = np.concatenate(
        [res.results[i]["loss"].reshape(PC) for i in range(N_CORES)]
    ).astype(np.float32)
    return loss, res


def kernel(targets: np.ndarray, params: np.ndarray,
           peopleIDs: np.ndarray | None = None) -> np.ndarray:
    loss, _ = run_spmd(targets, params, trace=False)
    return loss
